# revision 1
# baseline (speedup 1.0000x reference)
"""Trainium2 Bass kernel for the BaselinePreprocessor problem.

Computes, for full inputs:
  fused = concat([interp(vision->T), interp(proprio->T), imu], -1)  # [64,1024,550]
  vox_mean = mean(occupancy grid 64^3 of 10k points)               # scalar
  out = concat([fused, vox_mean bcast], -1)                        # [64,1024,551]

Strategy: pure data parallel over batch (8 cores x 8 batches). Linear
interpolation along time is a sparse linear map -> dense TensorE matmuls with
host-precomputed weight matrices (constants derived from shapes only). The
voxel histogram is built per-core via one indirect-DMA scatter of ones into a
DRAM grid, then reduced on-device.
"""

import numpy as np

import concourse.bacc as bacc
import concourse.bass as bass
import concourse.mybir as mybir
import concourse.tile as tile
from concourse.bass_utils import run_bass_kernel_spmd

F32 = mybir.dt.float32
F16 = mybir.dt.float16
BF16 = mybir.dt.bfloat16
I32 = mybir.dt.int32
ALU = mybir.AluOpType

N_CORES = 8
B_PER_CORE = 8
T = 1024
LV, CV = 64, 512     # vision input time-len, channels
LP, CP = 256, 32     # proprio
CI = 6               # imu channels (identity interp: L == T)
C_OUT = 551
GRID = 64
NVOX = GRID * GRID * GRID  # 262144
NPTS = 10000
NPTS_CORE = NPTS // N_CORES          # 1250 points scattered per core
PTS_P, PTS_F = 125, NPTS_CORE // 125  # [125, 10] per-core point layout
N_TILES = T // 128         # 8 time tiles of 128 rows


def _interp_weights_T(L: int) -> np.ndarray:
    """W^T [L, T] with W the [T, L] linear-interp matrix (align_corners)."""
    scale = np.float32((L - 1) / (T - 1))
    pos = np.arange(T, dtype=np.float32) * scale
    lo = np.clip(np.floor(pos).astype(np.int32), 0, L - 1)
    hi = np.minimum(lo + 1, L - 1)
    w = (pos - lo.astype(np.float32)).astype(np.float32)
    wt = np.zeros((L, T), dtype=np.float32)
    np.add.at(wt, (lo, np.arange(T)), np.float32(1.0) - w)
    np.add.at(wt, (hi, np.arange(T)), w)
    return np.ascontiguousarray(wt)


def _proprio_chunks_needed(j: int) -> list[int]:
    """Which K=128 row chunks of W_p^T have nonzeros for time tile j."""
    lo0 = (128 * j * (LP - 1)) // (T - 1)
    lo1 = (128 * j + 127) * (LP - 1) // (T - 1)
    hi1 = min(lo1 + 1, LP - 1)
    ks = []
    if lo0 < 128:
        ks.append(0)
    if hi1 >= 128:
        ks.append(1)
    return ks


def _emit(nc: bass.Bass, tc: tile.TileContext, ctx, debug_vox: bool = False):
    vision = nc.declare_dram_parameter("vision", [B_PER_CORE, LV, CV], F32, isOutput=False)
    proprio = nc.declare_dram_parameter("proprio", [B_PER_CORE, LP, CP], F32, isOutput=False)
    imu = nc.declare_dram_parameter("imu", [B_PER_CORE, T, CI], F32, isOutput=False)
    points = nc.declare_dram_parameter("points", [NPTS_CORE, 3], F32, isOutput=False)
    # vision interp weights in an fp16 hi/lo pair: W = wvh + wvl to ~2^-24
    # relative, so three fp16 matmuls (hi@hi + hi@lo + lo@hi) reproduce the
    # fp32 product to ~1e-6 absolute at far lower PE cost than fp32 matmul.
    wvh = nc.declare_dram_parameter("wvh", [LV, T], F16, isOutput=False)
    wvl = nc.declare_dram_parameter("wvl", [LV, T], F16, isOutput=False)
    wp = nc.declare_dram_parameter("wp", [LP, T], F32, isOutput=False)
    out = nc.declare_dram_parameter("out", [B_PER_CORE, T, C_OUT], F32, isOutput=True)

    # bf16 occupancy grid (0/1 values are exact; halves the AllReduce bytes)
    grid = nc.dram_tensor("grid", [NVOX, 1], BF16)
    grid_2d = grid[:].rearrange("(p f) o -> p (f o)", p=128)  # [128, 2048]
    grid_sh = nc.dram_tensor("grid_sh", [NVOX, 1], BF16, addr_space="Shared")
    grid_sh_2d = grid_sh[:].rearrange("(p f) o -> p (f o)", p=128)

    const = ctx.enter_context(tc.tile_pool(name="const", bufs=1))
    work = ctx.enter_context(tc.tile_pool(name="work", bufs=1))
    stream = ctx.enter_context(tc.tile_pool(name="stream", bufs=3))
    outp = ctx.enter_context(tc.tile_pool(name="outp", bufs=6))
    psumv = ctx.enter_context(tc.tile_pool(name="psumv", bufs=3, space="PSUM"))
    psump = ctx.enter_context(tc.tile_pool(name="psump", bufs=2, space="PSUM"))
    psums = ctx.enter_context(tc.tile_pool(name="psums", bufs=1, space="PSUM"))

    # ---------------- voxel occupancy scalar ----------------
    # zero the DRAM grid
    zer = const.tile([128, 2048], BF16)
    nc.vector.memset(zer[:], 0.0)
    nc.scalar.dma_start(out=grid_2d, in_=zer[:])

    # load points as [125, 80, 3]
    pts = work.tile([PTS_P, PTS_F, 3], F32)
    nc.scalar.dma_start(out=pts[:], in_=points[:].rearrange("(p f) c -> p f c", p=PTS_P))

    # per-coordinate voxel index, exactly replicating the reference arithmetic:
    # q = clip(trunc((p + 2) * 16), 0, 63); computed as clip-then-floor which
    # is equivalent (trunc==floor for the surviving non-negative range).
    # floor(x) for x in [0, 63]: round-trip through int32 (rounding mode of
    # the cast may be trunc or nearest) then subtract 1 wherever the result
    # exceeds x — exact either way.
    q = []
    ji = work.tile([PTS_P, PTS_F], I32)
    gt = work.tile([PTS_P, PTS_F], F32)
    for c in range(3):
        qc = work.tile([PTS_P, PTS_F], F32, tag=f"q{c}")
        nc.vector.tensor_scalar(qc[:], pts[:, :, c], 2.0, 16.0, ALU.add, ALU.mult)
        nc.vector.tensor_scalar(qc[:], qc[:], 63.0, 0.0, ALU.min, ALU.max)
        rt = work.tile([PTS_P, PTS_F], F32, tag=f"rt{c}")
        nc.vector.tensor_copy(out=ji[:], in_=qc[:])
        nc.vector.tensor_copy(out=rt[:], in_=ji[:])
        nc.vector.tensor_tensor(gt[:], rt[:], qc[:], ALU.is_gt)
        nc.vector.tensor_tensor(qc[:], rt[:], gt[:], ALU.subtract)
        q.append(qc)
    acc = work.tile([PTS_P, PTS_F], F32)
    nc.vector.tensor_scalar(acc[:], q[0][:], 64.0, None, ALU.mult)
    nc.vector.tensor_tensor(acc[:], acc[:], q[1][:], ALU.add)
    nc.vector.tensor_scalar(acc[:], acc[:], 64.0, None, ALU.mult)
    nc.vector.tensor_tensor(acc[:], acc[:], q[2][:], ALU.add)
    idx = work.tile([PTS_P, PTS_F], I32)
    nc.vector.tensor_copy(out=idx[:], in_=acc[:])  # exact integers -> exact

    # Scatter ones: the HW indirect DMA consumes ONE offset per partition
    # (writing the source's free dim contiguously there), so each call
    # scatters up to 128 points — one call per index column. Each core only
    # scatters its own 1/8 of the points; AllReduce(max) below unions the
    # partial occupancy grids.
    ones_pts = const.tile([PTS_P, 1], BF16)
    nc.vector.memset(ones_pts[:], 1.0)
    for f in range(PTS_F):
        nc.gpsimd.indirect_dma_start(
            out=grid[:],
            out_offset=bass.IndirectOffsetOnAxis(ap=idx[:, f:f + 1], axis=0),
            in_=ones_pts[:],
            in_offset=None,
        )
    nc.gpsimd.collective_compute(
        "AllReduce",
        ALU.max,
        replica_groups=[list(range(N_CORES))],
        ins=[grid[:]],
        outs=[grid_sh[:]],
    )

    if debug_vox:
        dbg_idx = nc.declare_dram_parameter("dbg_idx", [PTS_P, PTS_F], I32, isOutput=True)
        nc.sync.dma_start(out=dbg_idx[:], in_=idx[:])
        dbg_q = nc.declare_dram_parameter("dbg_q", [3, PTS_P, PTS_F], F32, isOutput=True)
        for c in range(3):
            nc.sync.dma_start(out=dbg_q[c], in_=q[c][:])

    # read back and reduce to the mean scalar, broadcast to [128,1]
    rb = work.tile([128, 2048], BF16)
    nc.scalar.dma_start(out=rb[:], in_=grid_sh_2d)

    if debug_vox:
        dbg_grid = nc.declare_dram_parameter("dbg_grid", [128, 2048], F32, isOutput=True)
        nc.sync.dma_start(out=dbg_grid[:], in_=rb[:])
    red = work.tile([128, 1], F32)
    nc.vector.tensor_reduce(red[:], rb[:], axis=mybir.AxisListType.X, op=ALU.add)
    ones_col = const.tile([128, 1], F32)
    nc.vector.memset(ones_col[:], 1.0)
    ps = psums.tile([1, 1], F32, tag="ps_scalar")
    nc.tensor.matmul(out=ps[:], lhsT=red[:], rhs=ones_col[:], start=True, stop=True)
    s_sb = work.tile([1, 1], F32)
    nc.vector.tensor_copy(out=s_sb[:], in_=ps[:])
    scale_row = const.tile([1, 128], F32)
    nc.vector.memset(scale_row[:], 1.0 / NVOX)  # 2**-18, exact
    pb = psums.tile([128, 1], F32, tag="ps_bcast")
    nc.tensor.matmul(out=pb[:], lhsT=scale_row[:], rhs=s_sb[:], start=True, stop=True)
    vox = work.tile([128, 1], F32)
    nc.vector.tensor_copy(out=vox[:], in_=pb[:])
    # The summary column is written by its own tiny per-batch DMAs so the
    # main output stream never waits on the voxel-scalar chain.
    vox_row = work.tile([128, N_TILES], F32)
    nc.vector.tensor_copy(out=vox_row[:], in_=vox[:].to_broadcast([128, N_TILES]))
    for b in range(B_PER_CORE):
        nc.sync.dma_start(
            out=out[b, :, 550:551].rearrange("(j p) o -> p (j o)", p=128),
            in_=vox_row[:],
        )

    # ---------------- interpolation via matmul ----------------
    wvh_sb = const.tile([LV, T], F16)
    nc.scalar.dma_start(out=wvh_sb[:], in_=wvh[:])
    wvl_sb = const.tile([LV, T], F16)
    nc.scalar.dma_start(out=wvl_sb[:], in_=wvl[:])
    wp0_sb = const.tile([128, T], F32)
    nc.scalar.dma_start(out=wp0_sb[:], in_=wp[0:128, :])
    wp1_sb = const.tile([128, T], F32)
    nc.scalar.dma_start(out=wp1_sb[:], in_=wp[128:256, :])
    wp_sb = [wp0_sb, wp1_sb]

    # all batches' proprio, laid out [k-row 128, chunk 2, batch 8, chan 32]:
    # one cross-batch matmul (N = 8*32) per (time tile, nonzero chunk).
    pall = const.tile([128, 2, B_PER_CORE, CP], F32)
    for k in range(2):
        nc.scalar.dma_start(
            out=pall[:, k, :, :],
            in_=proprio[:, 128 * k:128 * (k + 1), :].rearrange("b p c -> p b c"),
        )
    pp_tiles = []
    for j in range(N_TILES):
        js = slice(j * 128, (j + 1) * 128)
        ppj = psump.tile([128, B_PER_CORE, CP], F32, tag="pp")
        ks = _proprio_chunks_needed(j)
        for i, k in enumerate(ks):
            nc.tensor.matmul(
                out=ppj[:],
                lhsT=wp_sb[k][:, js],
                rhs=pall[:, k, :, :],
                start=(i == 0),
                stop=(i == len(ks) - 1),
            )
        pp_sb = work.tile([128, B_PER_CORE, CP], F32, tag=f"ppsb{j}", name=f"ppsb{j}")
        nc.vector.tensor_copy(out=pp_sb[:], in_=ppj[:])
        pp_tiles.append(pp_sb)

    for b in range(B_PER_CORE):
        vb = stream.tile([LV, CV], F32, tag="vb")
        nc.scalar.dma_start(out=vb[:], in_=vision[b])
        vh = stream.tile([LV, CV], F16, tag="vh")
        nc.vector.tensor_copy(out=vh[:], in_=vb[:])
        vtmp = stream.tile([LV, CV], F32, tag="vtmp")
        nc.vector.tensor_copy(out=vtmp[:], in_=vh[:])
        nc.vector.tensor_tensor(vtmp[:], vb[:], vtmp[:], ALU.subtract)
        vl = stream.tile([LV, CV], F16, tag="vl")
        nc.vector.tensor_copy(out=vl[:], in_=vtmp[:])
        ib = stream.tile([128, N_TILES, CI], F32, tag="ib")
        nc.scalar.dma_start(out=ib[:], in_=imu[b].rearrange("(j p) c -> p j c", j=N_TILES))

        for j in range(N_TILES):
            js = slice(j * 128, (j + 1) * 128)
            pv = psumv.tile([128, CV], F32, tag="pv")
            nc.tensor.matmul(out=pv[:], lhsT=wvh_sb[:, js], rhs=vh[:], start=True, stop=False)
            nc.tensor.matmul(out=pv[:], lhsT=wvh_sb[:, js], rhs=vl[:], start=False, stop=False)
            nc.tensor.matmul(out=pv[:], lhsT=wvl_sb[:, js], rhs=vh[:], start=False, stop=True)

            ob = outp.tile([128, 550], F32, tag="ob")
            nc.vector.tensor_copy(out=ob[:, 0:CV], in_=pv[:])
            nc.vector.tensor_copy(out=ob[:, CV:CV + CP], in_=pp_tiles[j][:, b, :])
            nc.vector.tensor_copy(out=ob[:, 544:550], in_=ib[:, j, :])
            nc.sync.dma_start(out=out[b, js, 0:550], in_=ob[:])


_CACHE: dict[str, object] = {}


def _get_nc() -> bass.Bass:
    if "nc" not in _CACHE:
        from contextlib import ExitStack

        # Bacc (not plain Bass): its finalize() legalizes sync waits (HW
        # allows at most one wait per instruction; extras are split into
        # event-semaphore instructions).
        nc = bacc.Bacc(None, num_devices=N_CORES)
        with ExitStack() as ctx:
            tc = ctx.enter_context(tile.TileContext(nc))
            _emit(nc, tc, ctx)
        if not nc.is_finalized():
            nc.finalize()
        _CACHE["nc"] = nc
    return _CACHE["nc"]  # type: ignore[return-value]


def _run(inputs: dict, trace: bool = False):
    vision = np.ascontiguousarray(np.asarray(inputs["vision"], dtype=np.float32))
    proprio = np.ascontiguousarray(np.asarray(inputs["proprio"], dtype=np.float32))
    imu = np.ascontiguousarray(np.asarray(inputs["imu"], dtype=np.float32))
    points = np.ascontiguousarray(np.asarray(inputs["points"], dtype=np.float32))
    wv = _interp_weights_T(LV)
    wvh = wv.astype(np.float16)
    wvl = (wv - wvh.astype(np.float32)).astype(np.float16)
    wp = _interp_weights_T(LP)

    nc = _get_nc()
    in_maps = []
    for i in range(N_CORES):
        sl = slice(i * B_PER_CORE, (i + 1) * B_PER_CORE)
        psl = slice(i * NPTS_CORE, (i + 1) * NPTS_CORE)
        in_maps.append({
            "vision": vision[sl],
            "proprio": proprio[sl],
            "imu": imu[sl],
            "points": np.ascontiguousarray(points[psl]),
            "wvh": wvh,
            "wvl": wvl,
            "wp": wp,
        })
    res = run_bass_kernel_spmd(nc, in_maps, list(range(N_CORES)), trace=trace)
    full = np.concatenate([res.results[i]["out"] for i in range(N_CORES)], axis=0)
    return full, res


def kernel(**inputs) -> np.ndarray:
    full, _ = _run(inputs)
    return full



# revision 2
# speedup vs baseline: 1.2232x; 1.2232x over previous
"""Trainium2 Bass kernel v3 for the BaselinePreprocessor problem.

Computes, for full inputs:
  fused = concat([interp(vision->T), interp(proprio->T), imu], -1)  # [64,1024,550]
  vox_mean = mean(occupancy grid 64^3 of 10k points)               # scalar
  out = concat([fused, vox_mean bcast], -1)                        # [64,1024,551]

Design (vs the fp32/fp16-3-term baseline at 181us):
- fp16 end-to-end for the dense stream: host casts inputs + interp weight
  matrices to fp16, device matmuls are single-term fp16 (tolerance is 2e-2;
  fp16 lands ~7e-4), and the dense output dram tensor is fp16 [B,1024,550]
  (halves HBM write traffic; host upcasts to fp32 after gather).
- time dim is permuted as t = 8p + j (p = SBUF partition, j = 0..7) via
  host-permuted weight matrices, so each batch's output tile [128, 8, 550]
  maps onto out[b] with partition stride 8*550 -> per-batch output is one
  fully-contiguous 1.1 MB DMA.
- PSUM drains are 2048-elem copies split across VectorE (jg=0) and ScalarE
  (jg=1); proprio for all 8 batches is one [128,8j,8b,32c] psum tile.
- voxels are sharded MOD-8 across cores (host routes each point by its
  z-bin mod 8 - pure shard routing; all voxel arithmetic re-done on device).
  Core i owns voxels v with v % 8 == i, so per-core occupancy sets are
  disjoint and the union count is the SUM of per-core counts: no collective
  at all. Each core scatters its ~1250 routed points into a private 64KB
  grid of u = (v - i)/8, reduces it, and returns count/NVOX as a [1,1]
  scalar; the host sums the 8 partials (exact: dyadic rationals) and
  broadcasts into column 550 (mirroring the reference's broadcast_to).
- the scatter is 12 indirect DMAs (128 offsets each - the HW max) that
  alternate between two grid copies so Tile's WAW hazard never serializes
  consecutive calls on DMA completion; the two copies are max-merged at
  readback.
"""

import numpy as np

import concourse.bacc as bacc
import concourse.bass as bass
import concourse.bass_isa as bass_isa
import concourse.mybir as mybir
import concourse.tile as tile
from concourse.bass_utils import run_bass_kernel_spmd

F32 = mybir.dt.float32
F16 = mybir.dt.float16
BF16 = mybir.dt.bfloat16
I32 = mybir.dt.int32
ALU = mybir.AluOpType

N_CORES = 8
B_PER_CORE = 8
T = 1024
LV, CV = 64, 512     # vision input time-len, channels
LP, CP = 256, 32     # proprio
CI = 6               # imu channels (identity interp: L == T)
C_DENSE = 550
GRID = 64
NVOX = GRID * GRID * GRID  # 262144
NLOC = NVOX // N_CORES     # 32768 voxels owned per core (v % 8 == core_id)
NPTS = 10000
PTS_F = 11                 # per-core routed-point capacity 128*11 = 1408
NJ = 8                     # time sub-index j; t = 8p + j


def _interp_weights_T(L: int) -> np.ndarray:
    """W^T [L, T] with W the [T, L] linear-interp matrix (align_corners)."""
    scale = np.float32((L - 1) / (T - 1))
    pos = np.arange(T, dtype=np.float32) * scale
    lo = np.clip(np.floor(pos).astype(np.int32), 0, L - 1)
    hi = np.minimum(lo + 1, L - 1)
    w = (pos - lo.astype(np.float32)).astype(np.float32)
    wt = np.zeros((L, T), dtype=np.float32)
    np.add.at(wt, (lo, np.arange(T)), np.float32(1.0) - w)
    np.add.at(wt, (hi, np.arange(T)), w)
    return wt


def _emit(nc: bass.Bass, tc: tile.TileContext, ctx):
    vis = nc.declare_dram_parameter("vis", [LV, B_PER_CORE, CV], F16, isOutput=False)
    prop = nc.declare_dram_parameter("prop", [128, 2, B_PER_CORE, CP], F16, isOutput=False)
    imu = nc.declare_dram_parameter("imu", [128, B_PER_CORE, NJ, CI], F16, isOutput=False)
    pts_d = nc.declare_dram_parameter("pts", [128, PTS_F, 3], F32, isOutput=False)
    wv = nc.declare_dram_parameter("wv", [LV, NJ, 128], F16, isOutput=False)
    wp = nc.declare_dram_parameter("wp", [128, 2, NJ, 128], F16, isOutput=False)
    out = nc.declare_dram_parameter("out", [B_PER_CORE, T, C_DENSE], F16, isOutput=True)
    vox_out = nc.declare_dram_parameter("vox", [1, 1], F32, isOutput=True)

    # private bf16 occupancy-grid copies (scatters round-robin; max-merged
    # at readback so consecutive scatters have no WAW hazard between them:
    # with 3 copies, call f's ~1.3us SWDGE emission time covers call f-3's
    # ~2.4us DMA completion, so the chain runs at emission rate)
    NGRID = 6
    grids = [nc.dram_tensor(f"grid{a}", [NLOC, 1], BF16) for a in range(NGRID)]
    grids_2d = [g[:].rearrange("(p f) o -> p (f o)", p=128) for g in grids]

    const = ctx.enter_context(tc.tile_pool(name="const", bufs=1))
    work = ctx.enter_context(tc.tile_pool(name="work", bufs=1))
    outp = ctx.enter_context(tc.tile_pool(name="outp", bufs=4))
    psum = ctx.enter_context(tc.tile_pool(name="psum", bufs=4, space="PSUM"))

    # ---------------- vox chain, part 1 (up-front) ----------------
    # all memset-fed constants go first in the vector stream: nothing they
    # gate (PE warm-up especially) should wait on an input DMA
    zer = const.tile([128, NLOC // 128], BF16)
    nc.vector.memset(zer[:], 0.0)
    ones_pts = const.tile([128, 1], BF16)
    nc.vector.memset(ones_pts[:], 1.0)
    inv_col = const.tile([128, 1], F32)
    nc.vector.memset(inv_col[:], 1.0 / NVOX)
    # ---------------- input loads (split across both HWDGE queues) --------
    # scalar queue: points first (gates the scatter chain), then the grid
    # zero-fills, then proprio operands (first real matmuls). sync queue:
    # vision/wv/imu (its first output write is not until ~16us, free ride).
    pts = work.tile([128, PTS_F, 3], F32)
    nc.scalar.dma_start(out=pts[:], in_=pts_d[:])
    prop_sb = const.tile([128, 2, B_PER_CORE, CP], F16)
    nc.scalar.dma_start(out=prop_sb[:], in_=prop[:])
    wp_sb = const.tile([128, 2, NJ, 128], F16)
    nc.scalar.dma_start(out=wp_sb[:], in_=wp[:])
    for a in range(NGRID):
        nc.scalar.dma_start(out=grids_2d[a], in_=zer[:])

    vis_sb = const.tile([LV, B_PER_CORE, CV], F16)
    nc.sync.dma_start(out=vis_sb[:], in_=vis[:])
    wv_sb = const.tile([LV, NJ, 128], F16)
    nc.sync.dma_start(out=wv_sb[:], in_=wv[:])
    imu_sb = const.tile([128, B_PER_CORE, NJ, CI], F16)
    nc.sync.dma_start(out=imu_sb[:], in_=imu[:])

    # ---------------- vox index math (vector) ----------------
    # q = clip(trunc((p + 2) * 16), 0, 63) per coord, exactly replicating the
    # reference: clip-then-floor == trunc-then-clip on the surviving range.
    # floor via int32 round-trip then subtract 1 where round-up occurred.
    q = []
    ji = work.tile([128, PTS_F], I32)
    gt = work.tile([128, PTS_F], F32)
    for c in range(3):
        qc = work.tile([128, PTS_F], F32, tag=f"q{c}")
        nc.vector.tensor_scalar(qc[:], pts[:, :, c], 2.0, 16.0, ALU.add, ALU.mult)
        nc.vector.tensor_scalar(qc[:], qc[:], 63.0, 0.0, ALU.min, ALU.max)
        rt = work.tile([128, PTS_F], F32, tag=f"rt{c}")
        nc.vector.tensor_copy(out=ji[:], in_=qc[:])
        nc.vector.tensor_copy(out=rt[:], in_=ji[:])
        nc.vector.tensor_tensor(gt[:], rt[:], qc[:], ALU.is_gt)
        nc.vector.tensor_tensor(qc[:], rt[:], gt[:], ALU.subtract)
        q.append(qc)
    # points are host-routed by key = (q0+q2) % 8, so this core's local
    # voxel index is u = (q0*64 + q1)*8 + floor(q2/8): a bijection from its
    # owned voxel set onto [0, 32768). floor(q2/8) via the same round-trip.
    m = work.tile([128, PTS_F], F32)
    nc.vector.tensor_scalar(m[:], q[2][:], 0.125, None, ALU.mult)
    mr = work.tile([128, PTS_F], F32)
    nc.vector.tensor_copy(out=ji[:], in_=m[:])
    nc.vector.tensor_copy(out=mr[:], in_=ji[:])
    nc.vector.tensor_tensor(gt[:], mr[:], m[:], ALU.is_gt)
    nc.vector.tensor_tensor(m[:], mr[:], gt[:], ALU.subtract)
    acc = work.tile([128, PTS_F], F32)
    nc.vector.tensor_scalar(acc[:], q[0][:], 64.0, None, ALU.mult)
    nc.vector.tensor_tensor(acc[:], acc[:], q[1][:], ALU.add)
    nc.vector.tensor_scalar(acc[:], acc[:], 8.0, None, ALU.mult)
    nc.vector.tensor_tensor(acc[:], acc[:], m[:], ALU.add)
    idx = work.tile([128, PTS_F], I32)
    nc.vector.tensor_copy(out=idx[:], in_=acc[:])  # exact integers -> exact

    # ---------------- scatter (gpsimd), alternating grid copies ----------
    for f in range(PTS_F):
        nc.gpsimd.indirect_dma_start(
            out=grids[f % NGRID][:],
            out_offset=bass.IndirectOffsetOnAxis(ap=idx[:, f:f + 1], axis=0),
            in_=ones_pts[:],
            in_offset=None,
        )
    # readbacks (gpsimd SWDGE, right after its scatters); the merge + count
    # ops are emitted later, interleaved into the vector stream mid-batch
    rb = [work.tile([128, NLOC // 128], BF16, tag=f"rb{a}", name=f"rb{a}")
          for a in range(NGRID)]
    for a in range(NGRID):
        nc.gpsimd.dma_start(out=rb[a][:], in_=grids_2d[a])
    red = work.tile([128, 1], F32)

    # proprio matmuls are emitted inside batch 0's vision-group loop below
    # (one j-half after each of b0's first two groups), so the PE reaches
    # batch 0's first drains ~7us sooner than a proprio-first ordering.
    pp_sb = work.tile([128, NJ, B_PER_CORE, CP], F16)

    def emit_proprio_half(h):
        pp_ps = psum.tile([128, NJ // 2, B_PER_CORE, CP], F32, tag="mm",
                          name=f"pp_ps{h}")
        for jh in range(NJ // 2):
            j = h * (NJ // 2) + jh
            for k in range(2):
                nc.tensor.matmul(
                    out=pp_ps[:, jh, :, :],
                    lhsT=wp_sb[:, k, j, :],
                    rhs=prop_sb[:, k, :, :],
                    start=(k == 0),
                    stop=(k == 1),
                )
        nc.vector.tensor_copy(
            out=pp_sb[:, h * (NJ // 2):(h + 1) * (NJ // 2), :, :], in_=pp_ps[:])

    # ---------------- vision matmuls + assembly per batch ----------------
    # each batch's output goes as TWO half-writes (time rows 8p+j for j<4 /
    # j>=4, each a fully-contiguous 2200B-per-partition chunk) so the write
    # stream starts as soon as the first j-half of batch 0 is assembled --
    # gated by proprio's first psum half rather than the whole batch.
    ob_v = out[:].rearrange("b (p j) c -> b p (j c)", p=128)
    for b in range(B_PER_CORE):
        ob = outp.tile([128, NJ, C_DENSE], F16, tag="ob")
        for half in range(2):
            for jg in range(2 * half, 2 * half + 2):
                vg = psum.tile([128, 2, CV], F32, tag="mm")
                for jj in range(2):
                    nc.tensor.matmul(
                        out=vg[:, jj, :],
                        lhsT=wv_sb[:, jg * 2 + jj, :],
                        rhs=vis_sb[:, b, :],
                        start=True,
                        stop=True,
                    )
                if jg % 2 == 0:
                    nc.vector.tensor_copy(
                        out=ob[:, jg * 2:jg * 2 + 2, 0:CV], in_=vg[:])
                else:
                    nc.scalar.copy(
                        out=ob[:, jg * 2:jg * 2 + 2, 0:CV], in_=vg[:])
                if b == 0 and jg % 2 == 1:
                    emit_proprio_half(jg // 2)
            jh = slice(4 * half, 4 * half + 4)
            nc.scalar.copy(out=ob[:, jh, 512:544], in_=pp_sb[:, jh, b, :])
            nc.scalar.copy(out=ob[:, jh, 544:550], in_=imu_sb[:, b, jh, :])
            nc.sync.dma_start(
                out=ob_v[b, :, half * 4 * C_DENSE:(half + 1) * 4 * C_DENSE],
                in_=ob[:, jh, :],
            )
    # vox merge + free-dim count at the tail of the vector stream (the
    # scatter chain + readbacks finish at about the same time as the drains)
    for a in range(1, NGRID):
        nc.vector.tensor_tensor(rb[0][:], rb[0][:], rb[a][:], ALU.max)
    nc.vector.tensor_reduce(
        red[:], rb[0][:], axis=mybir.AxisListType.X, op=ALU.add)

    # partition-sum on the (now idle) PE: vox = (1/NVOX ones)^T @ red
    vps = psum.tile([128, NJ // 2, B_PER_CORE, CP], F32, tag="mm")
    nc.tensor.matmul(out=vps[0:1, 0, 0, 0:1], lhsT=inv_col[:], rhs=red[:],
                     start=True, stop=True)
    vox_sb = work.tile([1, 1], F32)
    nc.vector.tensor_copy(out=vox_sb[:], in_=vps[0:1, 0, 0, 0:1])

    # vox scalar out (sync queue tail, after the big writes are enqueued)
    nc.sync.dma_start(out=vox_out[:], in_=vox_sb[:])


_CACHE: dict[str, object] = {}


def _get_nc() -> bass.Bass:
    if "nc" not in _CACHE:
        from contextlib import ExitStack

        nc = bacc.Bacc(None, num_devices=N_CORES)
        with ExitStack() as ctx:
            tc = ctx.enter_context(tile.TileContext(nc))
            _emit(nc, tc, ctx)
        if not nc.is_finalized():
            nc.finalize()
        _CACHE["nc"] = nc
    return _CACHE["nc"]  # type: ignore[return-value]


def _prep_weights():
    if "w" not in _CACHE:
        wt_v = _interp_weights_T(LV)   # [64, 1024]
        wt_p = _interp_weights_T(LP)   # [256, 1024]
        wv = np.ascontiguousarray(
            wt_v.reshape(LV, 128, NJ).transpose(0, 2, 1).astype(np.float16))
        wp = np.ascontiguousarray(
            wt_p.reshape(2, 128, 128, NJ).transpose(1, 0, 3, 2).astype(np.float16))
        _CACHE["w"] = (wv, wp)
    return _CACHE["w"]


def _route_points(points: np.ndarray) -> list[np.ndarray]:
    """Shard points by owning core = (x-bin + z-bin) % 8 (smoother than one
    coordinate's residue: the +-2 clamp mass convolves across bins); pad each
    shard to capacity with duplicates of its first point (union-preserving)."""
    n = min(NPTS, points.shape[0])
    pts = points[:n]
    q = np.clip(((pts + np.float32(2.0)) / np.float32(4.0)
                 * np.float32(GRID)).astype(np.int32), 0, GRID - 1)
    owner = (q[:, 0] + q[:, 2]) % N_CORES
    cap = 128 * PTS_F
    shards = []
    for i in range(N_CORES):
        p = pts[owner == i]
        assert 0 < p.shape[0] <= cap, f"core {i}: {p.shape[0]} points > cap {cap}"
        p = np.concatenate([p, np.broadcast_to(p[0:1], (cap - p.shape[0], 3))], 0)
        shards.append(np.ascontiguousarray(p.reshape(128, PTS_F, 3)))
    return shards


def _run(inputs: dict, trace: bool = False):
    vision = np.asarray(inputs["vision"], dtype=np.float32)
    proprio = np.asarray(inputs["proprio"], dtype=np.float32)
    imu = np.asarray(inputs["imu"], dtype=np.float32)
    points = np.asarray(inputs["points"], dtype=np.float32)
    wv, wp = _prep_weights()
    shards = _route_points(points)

    nc = _get_nc()
    in_maps = []
    for i in range(N_CORES):
        sl = slice(i * B_PER_CORE, (i + 1) * B_PER_CORE)
        in_maps.append({
            "vis": np.ascontiguousarray(
                vision[sl].astype(np.float16).transpose(1, 0, 2)),
            "prop": np.ascontiguousarray(
                proprio[sl].astype(np.float16).reshape(
                    B_PER_CORE, 2, 128, CP).transpose(2, 1, 0, 3)),
            "imu": np.ascontiguousarray(
                imu[sl].astype(np.float16).reshape(
                    B_PER_CORE, 128, NJ, CI).transpose(1, 0, 2, 3)),
            "pts": shards[i],
            "wv": wv,
            "wp": wp,
        })
    res = run_bass_kernel_spmd(nc, in_maps, list(range(N_CORES)), trace=trace)
    full = np.empty((N_CORES * B_PER_CORE, T, C_DENSE + 1), dtype=np.float32)
    for i in range(N_CORES):
        full[i * B_PER_CORE:(i + 1) * B_PER_CORE, :, 0:C_DENSE] = (
            res.results[i]["out"].astype(np.float32))
    vox = np.float32(0.0)
    for i in range(N_CORES):
        vox += np.float32(res.results[i]["vox"][0, 0])
    full[:, :, C_DENSE] = vox
    return full, res


def kernel(**inputs) -> np.ndarray:
    full, _ = _run(inputs)
    return full
